# revision 39
# baseline (speedup 1.0000x reference)
"""Trainium2 Bass kernel for nn_Attention_32263794328002.

Dense attention: x:[16,384,32,32], w_qkv:[1152,384], drop_mask:[16,6,1024,1024].
qkv = 1x1conv(x); per (b,h): attn = softmax(mask(qT k * scale)); out = attn @ v.

Strategy: pure data-parallel over batch (2 batches per core, 8 cores).
Per (b, h): S^T[m,n] = k^T q on the PE (contraction d=64; the two heads of a
pair are issued back-to-back at PE row offsets 0/64 so they co-execute in
disjoint row groups), exp on ScalarE (no max subtraction needed:
|scale*S| <~ 1.2), multiply by the bf16 complement mask on VectorE (2x mode),
then out2[d+1, n] = [v;1]^T @ p^T accumulated over m-tiles on the PE. The
65th output row is the softmax denominator; the host divides and fixes
layout.

Scheduling: attention is split into Sem units (S matmuls + exp + mask, needs
only the psS PSUM pool) and out2 units (needs psO). Phase 1 runs qkv for
both batches in a scoped psQ pool while Sem units for the first two head
pairs run concurrently, buffering p^T tiles. Phase 2 (psQ closed, psO open)
weaves the out2 backlog between the remaining Sem units so ScalarE never
stalls. PSUM: psS 4 banks always; psQ 4 banks phase 1; psO 4 banks phase 2.
"""

import sys

for _p in ("/opt/trn_rl_repo", "/opt/pypackages"):
    if _p not in sys.path:
        sys.path.append(_p)

import numpy as np
import ml_dtypes

import concourse.bass as bass  # noqa: F401
import concourse.bacc as bacc
import concourse.tile as tile
from concourse import mybir
from concourse.bass_utils import run_bass_kernel_spmd

BF16 = mybir.dt.bfloat16
F32 = mybir.dt.float32

B, C, H, W = 16, 384, 32, 32
HEADS = 6
D = C // HEADS          # 64
N = H * W               # 1024
NCORES = 8
BPC = B // NCORES       # batches per core = 2
HP = HEADS // 2         # head pairs = 3
MT = N // 128           # m tiles = 8
CT = C // 128           # contraction tiles for qkv = 3
SCALE = float(C) ** -0.5


def build_nc():
    nc = bacc.Bacc(None, target_bir_lowering=False, debug=False)

    x_d = nc.dram_tensor("x", [BPC, C, N], BF16, kind="ExternalInput")
    wT_d = nc.dram_tensor("wT", [C, 3 * C], BF16, kind="ExternalInput")
    mc_d = nc.dram_tensor("maskc", [BPC, HP, N, 2, N], BF16, kind="ExternalInput")
    out_d = nc.dram_tensor("out", [BPC, HEADS, D + 1, N], BF16, kind="ExternalOutput")

    with tile.TileContext(nc) as tc:
        with (
            tc.tile_pool(name="singles", bufs=1) as singles,
            tc.tile_pool(name="xpool", bufs=2) as xpool,
            tc.tile_pool(name="qkpool", bufs=2) as qkpool,
            tc.tile_pool(name="vpool", bufs=2) as vpool,
            tc.tile_pool(name="mcpool", bufs=6) as mcpool,
            tc.tile_pool(name="ppool", bufs=36) as ppool,
            tc.tile_pool(name="opool", bufs=4) as opool,
            tc.tile_pool(name="psS", bufs=2, space="PSUM") as psS,
        ):
            wT_sb = {}
            for ct in range(CT):
                wt = singles.tile([128, 3 * C], BF16, name=f"wT{ct}", tag=f"wT{ct}")
                nc.sync.dma_start(
                    out=wt[:, :], in_=wT_d[ct * 128 : (ct + 1) * 128, :]
                )
                wT_sb[ct] = wt

            x_sb = {}
            qk_sb = {}
            vT_sb = {}
            pts = {}

            def emit_x(b):
                engs = [nc.gpsimd, nc.scalar, nc.gpsimd]
                for ct in range(CT):
                    t = xpool.tile(
                        [128, N], BF16, name=f"x{b}_{ct}", tag=f"x{ct}"
                    )
                    engs[ct].dma_start(
                        out=t[:, :], in_=x_d[b, ct * 128 : (ct + 1) * 128, :]
                    )
                    x_sb[(b, ct)] = t

            def emit_qk(psQ, b, ot):
                # one q or k channel tile: ot 0..2 -> q c-tiles, 3..5 -> k.
                # ct outer / nh inner keeps same-weight matmuls adjacent so
                # the second needs no weight reload.
                t = qkpool.tile([128, N], BF16, name=f"qk{b}_{ot}", tag=f"qk{ot}")
                ps = [
                    psQ.tile([128, 512], F32, name=f"psqk{nh}", tag="psq")
                    for nh in range(2)
                ]
                for ct in range(CT):
                    for nh in range(2):
                        nc.tensor.matmul(
                            ps[nh][:, :],
                            wT_sb[ct][:, ot * 128 : (ot + 1) * 128],
                            x_sb[(b, ct)][:, nh * 512 : (nh + 1) * 512],
                            start=(ct == 0),
                            stop=(ct == CT - 1),
                        )
                        if ct == CT - 1:
                            nc.vector.tensor_copy(
                                out=t[:, nh * 512 : (nh + 1) * 512],
                                in_=ps[nh][:, :],
                            )
                qk_sb[(b, ot)] = t

            def emit_vT(psQ, b, nt):
                # v^T: [spatial m, c_v] with a ones column per head -> [m,h,65]
                ps = psQ.tile([128, 512], F32, name="psv", tag="psq")
                for ct in range(CT):
                    nc.tensor.matmul(
                        ps[:, 0:C],
                        x_sb[(b, ct)][:, nt * 128 : (nt + 1) * 128],
                        wT_sb[ct][:, 2 * C : 3 * C],
                        start=(ct == 0),
                        stop=(ct == CT - 1),
                    )
                t = vpool.tile(
                    [128, HEADS, D + 1], BF16, name=f"vT{b}_{nt}", tag=f"vT{nt}"
                )
                nc.vector.memset(t[:, :, D : D + 1], 1.0)
                nc.vector.tensor_copy(
                    out=t[:, :, 0:D],
                    in_=ps[:, 0:C].rearrange("p (h d) -> p h d", h=HEADS),
                )
                vT_sb[(b, nt)] = t

            def emit_sem(b, hp, mt):
                # S matmuls (head pair back-to-back per n-chunk for PE
                # row-group co-execution) + exp + mask multiply
                mc = mcpool.tile([128, 2, N], BF16, name="mc", tag="mc")
                nc.sync.dma_start(
                    out=mc[:, :, :],
                    in_=mc_d[b, hp, mt * 128 : (mt + 1) * 128, :, :],
                )
                # one PSUM tile per n-chunk holds BOTH heads (different
                # banks, shared recycle dependency) so the two 64-row
                # matmuls co-issue and co-execute in disjoint PE row groups
                pss = []
                for nh in range(2):
                    ps = psS.tile([128, N], F32, name=f"psS{nh}", tag="ps")
                    for j in range(2):
                        r0 = 64 * j
                        nc.tensor.matmul(
                            ps[:, j * 512 : (j + 1) * 512],
                            qk_sb[(b, 3 + hp)][
                                r0 : r0 + 64, mt * 128 : (mt + 1) * 128
                            ],
                            qk_sb[(b, hp)][r0 : r0 + 64, nh * 512 : (nh + 1) * 512],
                            start=True,
                            stop=True,
                            tile_position=(r0, 0),
                        )
                    pss.append(ps)
                for nh in range(2):
                    pT = ppool.tile([128, N], BF16, name=f"pT{nh}", tag="pT")
                    nc.scalar.activation(
                        out=pT[:, :],
                        in_=pss[nh][:, :],
                        func=mybir.ActivationFunctionType.Exp,
                        scale=SCALE,
                    )
                    nc.vector.tensor_mul(
                        pT.rearrange("p (j n) -> p j n", j=2),
                        pT.rearrange("p (j n) -> p j n", j=2),
                        mc[:, :, nh * 512 : (nh + 1) * 512],
                    )
                    pts[(b, hp, mt, nh)] = pT

            po_live = {}

            def emit_out2(psO, b, hp, mt):
                if (b, hp) not in po_live:
                    po_live[(b, hp)] = [
                        psO.tile([D + 1, N], F32, name=f"po{j}", tag=f"po{j}")
                        for j in range(2)
                    ]
                po = po_live[(b, hp)]
                for j in range(2):
                    for nh in range(2):
                        pT = pts[(b, hp, mt, nh)]
                        nc.tensor.matmul(
                            po[j][:, nh * 512 : (nh + 1) * 512],
                            vT_sb[(b, mt)][:, 2 * hp + j, :],
                            pT[:, j * 512 : (j + 1) * 512],
                            start=(mt == 0),
                            stop=(mt == MT - 1),
                            skip_group_check=True,
                        )
                for nh in range(2):
                    pts.pop((b, hp, mt, nh))
                if mt == MT - 1:
                    po = po_live.pop((b, hp))
                    last = b == BPC - 1 and hp == HP - 1
                    for j in range(2):
                        ob = opool.tile([D + 1, N], BF16, name="ob", tag="ob")
                        if last and j == 1:
                            nc.scalar.copy(out=ob[:, :], in_=po[j][:, :])
                        else:
                            nc.vector.tensor_copy(out=ob[:, :], in_=po[j][:, :])
                        nc.gpsimd.dma_start(out=out_d[b, 2 * hp + j], in_=ob[:, :])

            # ---- phase 1: qkv (psQ pool) woven with Sem units of the first
            # two head pairs of batch 0 ----
            sem_order = [(b, hp, mt) for b in range(BPC) for hp in range(HP)
                         for mt in range(MT)]
            out2_order = list(sem_order)
            si = 0
            with tc.tile_pool(name="psQ", bufs=4, space="PSUM") as psQ:
                emit_x(0)
                emit_qk(psQ, 0, 0)
                emit_qk(psQ, 0, 3)
                emit_x(1)
                qkv_items = (
                    [("qk", 0, 1), ("qk", 0, 4)]
                    + [("vT", 0, nt) for nt in range(MT)]
                    + [("qk", 0, 2), ("qk", 0, 5)]
                    + [("qk", 1, 0), ("qk", 1, 3)]
                    + [("vT", 1, nt) for nt in range(MT)]
                    + [("qk", 1, 1), ("qk", 1, 4), ("qk", 1, 2), ("qk", 1, 5)]
                )
                qi = 0
                while si < 2 * MT:  # hp0 + hp1 of batch 0
                    emit_sem(*sem_order[si])
                    si += 1
                    n_q = min(len(qkv_items), (len(qkv_items) * si) // 12) - qi
                    for _ in range(n_q):
                        kind, b_, i_ = qkv_items[qi]
                        (emit_qk if kind == "qk" else emit_vT)(psQ, b_, i_)
                        qi += 1

            # ---- phase 2: out2 backlog weaves between remaining Sem units
            # at 3 out2-units : 2 sem-units so the backlog drains to zero ----
            with tc.tile_pool(name="psO", bufs=1, space="PSUM") as psO:
                oi = 0
                tick = 0
                while oi < len(out2_order):
                    if si < len(sem_order):
                        emit_sem(*sem_order[si])
                        si += 1
                    # lead a new po pair with an extra sem so the PE does
                    # not stall on the previous pair's PSUM release
                    if (
                        oi < len(out2_order)
                        and out2_order[oi][2] == 0
                        and si < len(sem_order)
                    ):
                        emit_sem(*sem_order[si])
                        si += 1
                    n_o = 2 if tick % 2 == 0 else 1
                    if si >= len(sem_order):
                        n_o = len(out2_order) - oi
                    for _ in range(n_o):
                        if oi < len(out2_order) and oi < si:
                            emit_out2(psO, *out2_order[oi])
                            oi += 1
                    tick += 1

    nc.compile()
    return nc


_NC_CACHE = None


def _get_nc():
    global _NC_CACHE
    if _NC_CACHE is None:
        _NC_CACHE = build_nc()
    return _NC_CACHE


def prepare_in_maps(x, w_qkv, drop_mask):
    bf16 = ml_dtypes.bfloat16
    x_b = np.ascontiguousarray(x.reshape(B, C, N)).astype(bf16)
    wT = np.ascontiguousarray(w_qkv.T).astype(bf16)
    # complement mask -> [b, head_pair, m_key, head_in_pair, n_query]
    mcb = (~drop_mask.astype(bool)).reshape(B, HP, 2, N, N)
    mc = np.ascontiguousarray(mcb.transpose(0, 1, 4, 2, 3)).astype(bf16)
    in_maps = []
    for c in range(NCORES):
        sl = slice(c * BPC, (c + 1) * BPC)
        in_maps.append({"x": x_b[sl], "wT": wT, "maskc": mc[sl]})
    return in_maps


def postprocess(results):
    outs = []
    for c in range(NCORES):
        o = np.asarray(results[c]["out"]).astype(np.float32)  # [BPC, h, 65, n]
        num = o[:, :, :D, :]
        den = o[:, :, D : D + 1, :]
        outs.append((num / den).reshape(BPC, C, H, W))
    return np.concatenate(outs, axis=0)


def kernel(x, w_qkv, drop_mask):
    nc = _get_nc()
    in_maps = prepare_in_maps(np.asarray(x), np.asarray(w_qkv), np.asarray(drop_mask))
    res = run_bass_kernel_spmd(nc, in_maps, core_ids=list(range(NCORES)))
    return postprocess(res.results)


if __name__ == "__main__":
    rng = np.random.default_rng(0)
    x = rng.standard_normal((B, C, H, W), dtype=np.float32)
    w = rng.standard_normal((3 * C, C), dtype=np.float32) * 0.05
    m = rng.random((B, HEADS, N, N)) < 0.1
    out = kernel(x=x, w_qkv=w, drop_mask=m)
    print(out.shape, out.dtype)
